# revision 3
# baseline (speedup 1.0000x reference)
"""Self-contained Trainium2 kernel for the per-sample channel-attention layer.

Reference computation (per batch sample, with q = x reshaped [c, h*w]):
    energy = q @ q.T                  # [c, c]
    attn   = softmax(energy, axis=-1)
    out    = attn @ q                 # [c, h*w]
    out    = w2 @ out + b             # 1x1 conv = channel mixing

Strategy: data-parallel over batch (b=8) across 8 NeuronCores — attention is
fully per-sample, so there is no cross-core communication at all. Per core:

  Phase A: stream x (f32, 16 MB) from HBM in chunks; convert to bf16; build
           q^T tiles via PE transposes; accumulate energy = q q^T in PSUM.
  Phase B: softmax over key channels (rows of [256, 256] energy) → attn bf16.
           Then fold the 1x1 conv into the attention output matmul:
           w2 @ (attn @ q) == (w2 @ attn) @ q, so compute M^T = (w2 attn)^T
           = attn^T-free matmul from attn (natural layout) and w2^T.
  Phase CD: final = M @ q + b streamed over n in 512-wide chunks, f32 out.

All matmuls run in bf16 with f32 PSUM accumulation (the softmax logits are
dominated by the diagonal by ~16000, so bf16 energy is far more than enough
precision; the conv path sees ~0.3% relative error, well inside tolerance).
"""

import numpy as np

import concourse.bacc as bacc
import concourse.tile as tile
from concourse import mybir
from concourse.bass_utils import run_bass_kernel_spmd
from concourse.masks import make_identity

B, C, H, W = 8, 256, 128, 128
N = H * W            # 16384 pixels
NCORES = 8
ACH = 2048           # phase-A streaming chunk width (pixels)
NACH = N // ACH      # 8
SUB = 128            # PE transpose sub-block width
NSUB = ACH // SUB    # 16
DCH = 512            # phase-CD output chunk width (one f32 PSUM bank)
NDCH = N // DCH      # 32

F32 = mybir.dt.float32
BF16 = mybir.dt.bfloat16
AX = mybir.AxisListType.X
AF = mybir.ActivationFunctionType

_CACHE = {}


def _build():
    nc = bacc.Bacc(None, target_bir_lowering=False, debug=False)
    x_ext = nc.dram_tensor("x", [C, N], F32, kind="ExternalInput")
    w_ext = nc.dram_tensor("conv_w", [C, C], F32, kind="ExternalInput")
    b_ext = nc.dram_tensor("conv_b", [C, 1], F32, kind="ExternalInput")
    out_ext = nc.dram_tensor("out", [C, N], F32, kind="ExternalOutput")

    with tile.TileContext(nc) as tc:
        with (
            tc.tile_pool(name="const", bufs=1) as const,
            tc.tile_pool(name="qpool", bufs=1) as qpool,
            tc.tile_pool(name="stage", bufs=2) as stage,
            tc.tile_pool(name="small", bufs=2) as small,
            tc.tile_pool(name="qt", bufs=3) as qtp,
            tc.tile_pool(name="outp", bufs=3) as outp,
            tc.tile_pool(name="ps_t", bufs=2, space="PSUM") as ps_t,
            tc.tile_pool(name="ps_e", bufs=1, space="PSUM") as ps_e,
            tc.tile_pool(name="ps_cd", bufs=2, space="PSUM") as ps_cd,
        ):
            ident = const.tile([128, 128], BF16)
            make_identity(nc, ident)

            # conv weight: load [o, c] f32, convert bf16, PE-transpose to
            # w2T[cb] = [128 c_in, 256 o] tiles (lhsT layout for later matmuls).
            w2t = []
            wst = []
            for ob in range(2):
                wf = stage.tile([128, C], F32, tag=f"wf{ob}")
                nc.sync.dma_start(out=wf, in_=w_ext[ob * 128:(ob + 1) * 128, :])
                wb = stage.tile([128, C], BF16, tag=f"wb{ob}")
                nc.vector.tensor_copy(out=wb, in_=wf)
                wst.append(wb)
            for cb in range(2):
                wtp = ps_t.tile([128, 256], BF16, tag="tp")
                for ob in range(2):
                    nc.tensor.transpose(
                        wtp[:, ob * 128:(ob + 1) * 128],
                        wst[ob][:, cb * 128:(cb + 1) * 128],
                        ident,
                    )
                wt = const.tile([128, C], BF16, tag=f"w2t{cb}")
                nc.vector.tensor_copy(out=wt, in_=wtp)
                w2t.append(wt)

            bias = []
            for ob in range(2):
                bt = const.tile([128, 1], F32, tag=f"bias{ob}")
                nc.sync.dma_start(out=bt, in_=b_ext[ob * 128:(ob + 1) * 128, :])
                bias.append(bt)

            # ---- Phase A: stream x, build bf16 q + energy = q q^T ----
            e_ps = [
                ps_e.tile([128, C], F32, tag=f"e{ib}", name=f"e_ps{ib}")
                for ib in range(2)
            ]
            qtiles = []  # per chunk: (qc0, qc1) bf16 [128, ACH]
            for ci in range(NACH):
                sl = slice(ci * ACH, (ci + 1) * ACH)
                xf0 = stage.tile([128, ACH], F32, tag="xf0")
                nc.sync.dma_start(out=xf0, in_=x_ext[0:128, sl])
                xf1 = stage.tile([128, ACH], F32, tag="xf1")
                nc.sync.dma_start(out=xf1, in_=x_ext[128:256, sl])
                qc0 = qpool.tile([128, ACH], BF16, tag=f"q0_{ci}")
                qc1 = qpool.tile([128, ACH], BF16, tag=f"q1_{ci}")
                nc.vector.tensor_copy(out=qc0, in_=xf0)
                nc.gpsimd.tensor_copy(out=qc1, in_=xf1)
                qtiles.append((qc0, qc1))

                for s in range(NSUB):
                    ssl = slice(s * SUB, (s + 1) * SUB)
                    tp = ps_t.tile([128, 256], BF16, tag="tp")
                    nc.tensor.transpose(tp[:, 0:128], qc0[:, ssl], ident)
                    nc.tensor.transpose(tp[:, 128:256], qc1[:, ssl], ident)
                    qt = qtp.tile([128, 256], BF16, tag="qt")
                    nc.vector.tensor_copy(out=qt[:, 0:128], in_=tp[:, 0:128])
                    nc.scalar.copy(out=qt[:, 128:256], in_=tp[:, 128:256])
                    first = ci == 0 and s == 0
                    last = ci == NACH - 1 and s == NSUB - 1
                    for ib in range(2):
                        nc.tensor.matmul(
                            e_ps[ib],
                            qt[:, ib * 128:(ib + 1) * 128],
                            qt[:, :],
                            start=first,
                            stop=last,
                            skip_group_check=True,
                        )

            # ---- Phase B: softmax rows of energy → attn (bf16, natural) ----
            attn = []
            for ib in range(2):
                nmx = small.tile([128, 1], F32, tag=f"nmx{ib}")
                nc.vector.reduce_max(out=nmx, in_=e_ps[ib], axis=AX, negate=True)
                pex = small.tile([128, C], BF16, tag=f"pex{ib}")
                ssum = small.tile([128, 1], F32, tag=f"ssum{ib}")
                nc.scalar.activation(
                    out=pex, in_=e_ps[ib], func=AF.Exp,
                    bias=nmx, scale=1.0, accum_out=ssum,
                )
                rec = small.tile([128, 1], F32, tag=f"rec{ib}")
                nc.vector.reciprocal(out=rec, in_=ssum)
                at = small.tile([128, C], BF16, tag=f"attn{ib}")
                nc.vector.tensor_scalar_mul(out=at, in0=pex, scalar1=rec)
                attn.append(at)

            # M^T = (w2 @ attn)^T = attn^T w2^T: out[j, o] from lhsT=attn
            # (natural [i, j]) and rhs = w2T [i, o]; fold conv into attention.
            mt = []
            for jb in range(2):
                jsl = slice(jb * 128, (jb + 1) * 128)
                mtp = ps_t.tile([128, C], F32, tag="tp")
                nc.tensor.matmul(mtp, attn[0][:, jsl], w2t[0][:, :],
                                 start=True, stop=False)
                nc.tensor.matmul(mtp, attn[1][:, jsl], w2t[1][:, :],
                                 start=False, stop=True)
                mts = small.tile([128, C], BF16, tag=f"mt{jb}")
                nc.vector.tensor_copy(out=mts, in_=mtp)
                mt.append(mts)

            # ---- Phase CD: final = M @ q + b, streamed over pixels ----
            for cj in range(NDCH):
                ci, off = divmod(cj * DCH, ACH)
                qc0, qc1 = qtiles[ci]
                q0s = qc0[:, off:off + DCH]
                q1s = qc1[:, off:off + DCH]
                sl = slice(cj * DCH, (cj + 1) * DCH)
                fp = ps_cd.tile([128, 2, DCH], F32, tag="fp")
                for ob in range(2):
                    osl = slice(ob * 128, (ob + 1) * 128)
                    nc.tensor.matmul(fp[:, ob, :], mt[0][:, osl], q0s,
                                     start=True, stop=False)
                    nc.tensor.matmul(fp[:, ob, :], mt[1][:, osl], q1s,
                                     start=False, stop=True)
                f0 = outp.tile([128, DCH], F32, tag="f0")
                nc.vector.tensor_scalar_add(out=f0, in0=fp[:, 0, :],
                                            scalar1=bias[0])
                nc.sync.dma_start(out=out_ext[0:128, sl], in_=f0)
                f1 = outp.tile([128, DCH], F32, tag="f1")
                nc.scalar.add(out=f1, in_=fp[:, 1, :], add=bias[1])
                nc.sync.dma_start(out=out_ext[128:256, sl], in_=f1)

    nc.compile()
    return nc


def _get_nc():
    if "nc" not in _CACHE:
        _CACHE["nc"] = _build()
    return _CACHE["nc"]


def kernel(x, conv_w, conv_b):
    x = np.ascontiguousarray(np.asarray(x), dtype=np.float32)
    w2 = np.ascontiguousarray(np.asarray(conv_w, dtype=np.float32)[:, :, 0, 0])
    bb = np.ascontiguousarray(np.asarray(conv_b, dtype=np.float32).reshape(C, 1))
    nc = _get_nc()
    in_maps = [
        {"x": np.ascontiguousarray(x[i].reshape(C, N)), "conv_w": w2, "conv_b": bb}
        for i in range(B)
    ]
    res = run_bass_kernel_spmd(nc, in_maps, core_ids=list(range(NCORES)))
    out = np.stack(
        [res.results[i]["out"].reshape(C, H, W) for i in range(B)], axis=0
    )
    return out
